# revision 1
# baseline (speedup 1.0000x reference)
"""BinaryDense kernel for Trainium2: out = sign(x) @ sign(w).

Full shapes: x [8192, 4096] f32, w [4096, 4096] f32 -> out [8192, 4096] f32.

Sharding over 8 NeuronCores (2D): x rows split 4 ways, w columns split 2 ways.
Each core computes a [2048, 2048] output block from x_shard [2048, 4096] and
w_shard [4096, 2048]. The host slices inputs and reassembles the output; no
collectives are needed.

Per-core kernel: sign-binarize both operands to bf16 on-chip (+-1 is exact in
bf16, products are +-1 and sums are integers <= 4096, so fp32 PSUM
accumulation is exact), keep the binarized w resident in SBUF, transpose
sign(x) tiles with the PE transposer, and run bf16 matmuls.
"""

import numpy as np

import concourse.mybir as mybir
import concourse.tile as tile
from concourse import bacc
from concourse.bass_utils import run_bass_kernel_spmd
from concourse.masks import make_identity

P = 128
N_CORES = 8
RM, RN = 4, 2            # row shards of x, column shards of w
M_FULL, K, N_FULL = 8192, 4096, 4096
M_SH, N_SH = M_FULL // RM, N_FULL // RN   # 2048, 2048
KB = K // P              # 32 contraction tiles
MB = M_SH // P           # 16 row blocks
NB = N_SH // 512         # 4 psum-width column chunks

F32 = mybir.dt.float32
BF16 = mybir.dt.bfloat16

_NC_CACHE = None


def build_nc():
    nc = bacc.Bacc("TRN2", target_bir_lowering=False, debug=False,
                   num_devices=N_CORES)
    x = nc.dram_tensor("x", [M_SH, K], F32, kind="ExternalInput").ap()
    w = nc.dram_tensor("w", [K, N_SH], F32, kind="ExternalInput").ap()
    out = nc.dram_tensor("out", [M_SH, N_SH], F32, kind="ExternalOutput").ap()

    with tile.TileContext(nc) as tc:
        with (
            tc.tile_pool(name="const", bufs=1) as const_pool,
            tc.tile_pool(name="wbin", bufs=1) as wbin_pool,
            tc.tile_pool(name="ftmp", bufs=2) as ftmp_pool,
            tc.tile_pool(name="xbin", bufs=2) as xbin_pool,
            tc.tile_pool(name="xT", bufs=2) as xT_pool,
            tc.tile_pool(name="obuf", bufs=2) as obuf_pool,
            tc.tile_pool(name="psumT", bufs=2, space="PSUM") as psumT_pool,
            tc.tile_pool(name="psumO", bufs=2, space="PSUM") as psumO_pool,
        ):
            ident = const_pool.tile([P, P], BF16)
            make_identity(nc, ident)

            # Binarized w, resident for the whole kernel: [128, kb, n]
            wbin = wbin_pool.tile([P, KB, N_SH], BF16)
            for kb in range(KB):
                wt = ftmp_pool.tile([P, N_SH], F32, tag="ftmp")
                nc.sync.dma_start(out=wt[:], in_=w[kb * P:(kb + 1) * P, :])
                nc.scalar.sign(wbin[:, kb, :], wt[:])

            for mb in range(MB):
                # Load one 128-row block of x and binarize to bf16.
                xb = xbin_pool.tile([P, K], BF16)
                for h in range(2):
                    xt = ftmp_pool.tile([P, K // 2], F32, tag="ftmp")
                    nc.sync.dma_start(
                        out=xt[:],
                        in_=x[mb * P:(mb + 1) * P, h * (K // 2):(h + 1) * (K // 2)])
                    nc.scalar.sign(xb[:, h * (K // 2):(h + 1) * (K // 2)], xt[:])

                # PE-transpose the 32 [128,128] tiles: xT[:, kb, m] = xb[m, kb*128+*]
                xT = xT_pool.tile([P, KB, P], BF16)
                for kb in range(KB):
                    pt = psumT_pool.tile([P, P], BF16)
                    nc.tensor.transpose(pt[:], xb[:, kb * P:(kb + 1) * P], ident[:])
                    nc.vector.tensor_copy(out=xT[:, kb, :], in_=pt[:])

                # Matmuls: accumulate over kb into one psum bank per 512-wide chunk.
                for nb in range(NB):
                    po = psumO_pool.tile([P, 512], F32)
                    for kb in range(KB):
                        nc.tensor.matmul(
                            po[:], xT[:, kb, :],
                            wbin[:, kb, nb * 512:(nb + 1) * 512],
                            start=(kb == 0), stop=(kb == KB - 1))
                    ob = obuf_pool.tile([P, 512], F32, tag="obuf")
                    nc.any.tensor_copy(out=ob[:], in_=po[:])
                    nc.sync.dma_start(
                        out=out[mb * P:(mb + 1) * P, nb * 512:(nb + 1) * 512],
                        in_=ob[:])

    nc.compile()
    return nc


def get_nc():
    global _NC_CACHE
    if _NC_CACHE is None:
        _NC_CACHE = build_nc()
    return _NC_CACHE


def kernel(x: np.ndarray, w: np.ndarray) -> np.ndarray:
    x = np.asarray(x, dtype=np.float32)
    w = np.asarray(w, dtype=np.float32)
    assert x.shape == (M_FULL, K) and w.shape == (K, N_FULL)

    nc = get_nc()
    in_maps = []
    for c in range(N_CORES):
        mi, ni = divmod(c, RN)
        in_maps.append({
            "x": np.ascontiguousarray(x[mi * M_SH:(mi + 1) * M_SH, :]),
            "w": np.ascontiguousarray(w[:, ni * N_SH:(ni + 1) * N_SH]),
        })
    res = run_bass_kernel_spmd(nc, in_maps, list(range(N_CORES)))

    out = np.empty((M_FULL, N_FULL), dtype=np.float32)
    for c in range(N_CORES):
        mi, ni = divmod(c, RN)
        out[mi * M_SH:(mi + 1) * M_SH, ni * N_SH:(ni + 1) * N_SH] = \
            res.results[c]["out"]
    return out


# revision 39
# speedup vs baseline: 95310.2482x; 95310.2482x over previous
"""BinaryDense kernel for Trainium2: out = sign(x) @ sign(w).

Full shapes: x [8192, 4096] f32, w [4096, 4096] f32 -> out [8192, 4096] f32.

Sharding over 8 NeuronCores (2D): x rows split 4 ways, w columns split 2 ways.
Each core computes a [2048, 2048] output block from x_shard [2048, 4096] and
w_shard [4096, 2048]. The host slices inputs and reassembles the output; no
collectives are needed.

Per-core kernel: binarize both operands on-chip to fp8e4 (+-1 is exact,
products are +-1 and sums are integers <= 4096, so fp32 PSUM accumulation is
exact), keep binarized w resident in SBUF, and run fp8 DoubleRow matmuls
(2 contraction tiles per pass).

Input handling:
  - Loads are SWDGE casting DMAs (f32 DRAM -> bf16 SBUF).  bf16 rounding
    cannot flip a sign (values below the smallest bf16 subnormal would need
    |x| < 5e-41; probability ~0 for randn inputs), so sign() is unaffected.
  - x tiles are PE-transposed directly out of the bf16 staging buffer; the
    sign binarization is FUSED into the PSUM->SBUF eviction (ACT Sign
    activation, bf16 psum -> fp8 SBUF).  No separate sign pass over x.
  - w signs run on the otherwise-idle GpSimd engine (min/max-clamp sign,
    step 2 on DVE).

Scheduling: DMA order is X_HEAD x blocks, all of w, then the remaining x
blocks; each m-block is one PE burst (32 transposes emitted TLOOK blocks
ahead + 64 DoubleRow matmuls into one psum bank per 512-wide chunk, full-K
accumulation, single eviction copy per bank on DVE).
"""

import numpy as np

import concourse.mybir as mybir
import concourse.tile as tile
from concourse import bacc
from concourse.bass_utils import run_bass_kernel_spmd
from concourse.masks import make_identity

P = 128
N_CORES = 8
RM, RN = 4, 2            # row shards of x, column shards of w
M_FULL, K, N_FULL = 8192, 4096, 4096
M_SH, N_SH = M_FULL // RM, N_FULL // RN   # 2048, 2048
KB = K // P              # 32 contraction tiles
MB = M_SH // P           # 16 row blocks
NB = N_SH // 512         # 4 psum-width column chunks
TLOOK = 2                # transpose lookahead (m-blocks) over matmuls
X_HEAD = 2               # x blocks loaded before w
TGRP = 16                # transposes sharing one psum tile (bf16: 2 banks)
KSPLIT = 1               # contraction pieces (psum accumulation groups)

USE_FP8_DR = True

F32 = mybir.dt.float32
BF16 = mybir.dt.bfloat16
FP8 = mybir.dt.float8e4
I16 = mybir.dt.int16

_NC_CACHE = None


def build_nc():
    mm_dt = FP8 if USE_FP8_DR else BF16

    nc = bacc.Bacc("TRN2", target_bir_lowering=False, debug=False,
                   num_devices=N_CORES)
    x = nc.dram_tensor("x", [M_SH, K], F32, kind="ExternalInput").ap()
    w = nc.dram_tensor("w", [K, N_SH], F32, kind="ExternalInput").ap()
    out = nc.dram_tensor("out", [M_SH, N_SH], I16, kind="ExternalOutput").ap()

    with tile.TileContext(nc) as tc:
        with (
            tc.tile_pool(name="const", bufs=1) as const_pool,
            tc.tile_pool(name="wbin", bufs=1) as wbin_pool,
            tc.tile_pool(name="xTr", bufs=1) as xT_pool,
            tc.tile_pool(name="ftmp", bufs=4) as ftmp_pool,
            tc.tile_pool(name="obuf", bufs=2) as obuf_pool,
            tc.tile_pool(name="psumT", bufs=2, space="PSUM") as psumT_pool,
            tc.tile_pool(name="psumO", bufs=4, space="PSUM") as psumO_pool,
        ):
            ident = const_pool.tile([P, P], BF16)
            make_identity(nc, ident)

            # Binarized, resident operands: w as [p, kb, n]; xT as [p, mb, kb, m]
            wbin = wbin_pool.tile([P, KB, N_SH], mm_dt)
            xT = xT_pool.tile([P, MB, KB, P], mm_dt)
            xstage = [None] * MB

            def load_x(mb):
                xt = ftmp_pool.tile([P, K], BF16, tag="xstage")
                nc.gpsimd.dma_start(out=xt[:], in_=x[mb * P:(mb + 1) * P, :])
                xstage[mb] = xt

            def load_w(kb):
                wt = ftmp_pool.tile([P, N_SH], BF16, tag="wstage")
                nc.gpsimd.dma_start(out=wt[:], in_=w[kb * P:(kb + 1) * P, :])
                dst = wbin[:, kb, :]
                # w signs alternate ACT (1-op Sign) / DVE (2-op clamp sign:
                # min(max(x*HUGE,-1),1), exact except |x| below the smallest
                # bf16 subnormal, probability ~0 for randn inputs; sign(0)=0
                # is preserved) so no single engine paces the w pipeline.
                if kb % 2 == 0:
                    nc.scalar.sign(dst, wt[:])
                else:
                    nc.vector.tensor_scalar(
                        dst, wt[:], 3.4e38, -1.0,
                        mybir.AluOpType.mult, mybir.AluOpType.max)
                    nc.vector.tensor_scalar(
                        dst, dst, 1.0, None, mybir.AluOpType.min)

            def transposes(mb):
                xt = xstage[mb]
                for g in range(KB // TGRP):
                    pt = psumT_pool.tile([P, TGRP, P], BF16, tag="psumT")
                    for j in range(TGRP):
                        kb = g * TGRP + j
                        nc.tensor.transpose(
                            pt[:, j, :], xt[:, kb * P:(kb + 1) * P], ident[:])
                    # Fused sign + downconvert during PSUM eviction.
                    nc.scalar.sign(xT[:, mb, g * TGRP:(g + 1) * TGRP, :], pt[:])

            def matmuls(mb):
                # Output staged as int16 (all values are integers <= 4096,
                # exactly representable); halves the output DMA bytes.  The
                # host widens back to f32.
                ob = obuf_pool.tile([P, N_SH], I16, tag="obuf")
                for nb in range(NB):
                    nsl = slice(nb * 512, (nb + 1) * 512)
                    for piece in range(KSPLIT):
                        po = psumO_pool.tile([P, 512], F32, tag="psumO")
                        kbs = range(piece * (KB // KSPLIT),
                                    (piece + 1) * (KB // KSPLIT))
                        if USE_FP8_DR:
                            pairs = list(kbs)[::2]
                            for i, kb in enumerate(pairs):
                                nc.tensor.matmul(
                                    po[:], xT[:, mb, kb:kb + 2, :],
                                    wbin[:, kb:kb + 2, nsl],
                                    start=(i == 0), stop=(i == len(pairs) - 1),
                                    perf_mode=mybir.MatmulPerfMode.DoubleRow)
                        else:
                            for i, kb in enumerate(kbs):
                                nc.tensor.matmul(
                                    po[:], xT[:, mb, kb, :], wbin[:, kb, nsl],
                                    start=(i == 0), stop=(i == KB // KSPLIT - 1))
                        if piece == 0:
                            nc.vector.tensor_copy(out=ob[:, nsl], in_=po[:])
                        else:
                            nc.vector.tensor_add(
                                out=ob[:, nsl], in0=po[:], in1=ob[:, nsl])
                nc.sync.dma_start(out=out[mb * P:(mb + 1) * P, :], in_=ob[:])

            # DMA issue order: a few x blocks, then all of w, then the rest
            # of x streaming behind the PE.
            for mb in range(X_HEAD):
                load_x(mb)
            for kb in range(KB):
                load_w(kb)
            for mb in range(X_HEAD, MB):
                load_x(mb)

            # Compute: transposes run TLOOK m-blocks ahead of matmuls.
            for mb in range(TLOOK):
                transposes(mb)
            for mb in range(MB):
                matmuls(mb)
                if mb + TLOOK < MB:
                    transposes(mb + TLOOK)

    nc.compile()
    return nc


def get_nc():
    global _NC_CACHE
    if _NC_CACHE is None:
        _NC_CACHE = build_nc()
    return _NC_CACHE


def kernel(x: np.ndarray, w: np.ndarray) -> np.ndarray:
    x = np.asarray(x, dtype=np.float32)
    w = np.asarray(w, dtype=np.float32)
    assert x.shape == (M_FULL, K) and w.shape == (K, N_FULL)

    nc = get_nc()
    in_maps = []
    for c in range(N_CORES):
        mi, ni = divmod(c, RN)
        in_maps.append({
            "x": np.ascontiguousarray(x[mi * M_SH:(mi + 1) * M_SH, :]),
            "w": np.ascontiguousarray(w[:, ni * N_SH:(ni + 1) * N_SH]),
        })
    res = run_bass_kernel_spmd(nc, in_maps, list(range(N_CORES)))

    out = np.empty((M_FULL, N_FULL), dtype=np.float32)
    for c in range(N_CORES):
        mi, ni = divmod(c, RN)
        out[mi * M_SH:(mi + 1) * M_SH, ni * N_SH:(ni + 1) * N_SH] = \
            res.results[c]["out"].astype(np.float32)
    return out


# revision 48
# speedup vs baseline: 96711.5334x; 1.0147x over previous
"""BinaryDense kernel for Trainium2: out = sign(x) @ sign(w).

Full shapes: x [8192, 4096] f32, w [4096, 4096] f32 -> out [8192, 4096] f32.

Sharding over 8 NeuronCores (2D): x rows split 4 ways, w columns split 2 ways.
Each core computes a [2048, 2048] output block from x_shard [2048, 4096] and
w_shard [4096, 2048]. The host slices inputs and reassembles the output; no
collectives are needed.

Per-core kernel: binarize both operands on-chip to fp8e4 (+-1 is exact,
products are +-1 and sums are integers <= 4096, so fp32 PSUM accumulation is
exact), keep binarized w resident in SBUF, and run fp8 DoubleRow matmuls
(2 contraction tiles per pass).

Input handling:
  - Loads are SWDGE casting DMAs (f32 DRAM -> bf16 SBUF).  bf16 rounding
    cannot flip a sign (values below the smallest bf16 subnormal would need
    |x| < 5e-41; probability ~0 for randn inputs), so sign() is unaffected.
  - x tiles are PE-transposed directly out of the bf16 staging buffer; the
    sign binarization is FUSED into the PSUM->SBUF eviction (ACT Sign
    activation, bf16 psum -> fp8 SBUF).  No separate sign pass over x.
  - w signs run on the otherwise-idle GpSimd engine (min/max-clamp sign,
    step 2 on DVE).

Scheduling: DMA order is X_HEAD x blocks, all of w, then the remaining x
blocks; each m-block is one PE burst (32 transposes emitted TLOOK blocks
ahead + 64 DoubleRow matmuls into one psum bank per 512-wide chunk, full-K
accumulation, single eviction copy per bank on DVE).
"""

import numpy as np

import concourse.mybir as mybir
import concourse.tile as tile
from concourse import bacc
from concourse.bass_utils import run_bass_kernel_spmd
from concourse.masks import make_identity

P = 128
N_CORES = 8
RM, RN = 4, 2            # row shards of x, column shards of w
M_FULL, K, N_FULL = 8192, 4096, 4096
M_SH, N_SH = M_FULL // RM, N_FULL // RN   # 2048, 2048
KB = K // P              # 32 contraction tiles
MB = M_SH // P           # 16 row blocks
NB = N_SH // 512         # 4 psum-width column chunks
TLOOK = 2                # transpose lookahead (m-blocks) over matmuls
X_HEAD = 2               # x blocks loaded before w
TGRP = 8                 # transposes sharing one psum tile (bf16: 1 bank)
KSPLIT = 1               # contraction pieces (psum accumulation groups)

USE_FP8_DR = True

F32 = mybir.dt.float32
BF16 = mybir.dt.bfloat16
FP8 = mybir.dt.float8e4
I16 = mybir.dt.int16

_NC_CACHE = None


def build_nc():
    mm_dt = FP8 if USE_FP8_DR else BF16

    nc = bacc.Bacc("TRN2", target_bir_lowering=False, debug=False,
                   num_devices=N_CORES)
    x = nc.dram_tensor("x", [M_SH, K], F32, kind="ExternalInput").ap()
    w = nc.dram_tensor("w", [K, N_SH], F32, kind="ExternalInput").ap()
    out = nc.dram_tensor("out", [M_SH, N_SH], I16, kind="ExternalOutput").ap()

    with tile.TileContext(nc) as tc:
        with (
            tc.tile_pool(name="const", bufs=1) as const_pool,
            tc.tile_pool(name="wbin", bufs=1) as wbin_pool,
            tc.tile_pool(name="xTr", bufs=1) as xT_pool,
            tc.tile_pool(name="ftmp", bufs=4) as ftmp_pool,
            tc.tile_pool(name="obuf", bufs=3) as obuf_pool,
            tc.tile_pool(name="psumT", bufs=3, space="PSUM") as psumT_pool,
            tc.tile_pool(name="psumO", bufs=5, space="PSUM") as psumO_pool,
        ):
            ident = const_pool.tile([P, P], BF16)
            make_identity(nc, ident)

            # Binarized, resident operands: w as [p, kb, n]; xT as [p, mb, kb, m]
            wbin = wbin_pool.tile([P, KB, N_SH], mm_dt)
            xT = xT_pool.tile([P, MB, KB, P], mm_dt)
            xstage = [None] * MB

            def load_x(mb):
                xt = ftmp_pool.tile([P, K], BF16, tag="xstage")
                nc.gpsimd.dma_start(out=xt[:], in_=x[mb * P:(mb + 1) * P, :])
                xstage[mb] = xt

            w3d = w.rearrange("(o p) n -> p o n", p=P)   # [128, KB, N_SH]

            def load_w(kb2):
                # Load two k-tiles per DMA (1 MiB destination).
                wt = ftmp_pool.tile([P, 2, N_SH], BF16, tag="wstage")
                nc.gpsimd.dma_start(
                    out=wt[:], in_=w3d[:, 2 * kb2:2 * kb2 + 2, :])
                dst = wbin[:, 2 * kb2:2 * kb2 + 2, :]
                # w signs alternate ACT (1-op Sign) / DVE (2-op clamp sign:
                # min(max(x*HUGE,-1),1), exact except |x| below the smallest
                # bf16 subnormal, probability ~0 for randn inputs; sign(0)=0
                # is preserved) so no single engine paces the w pipeline.
                if kb2 % 2 == 0:
                    nc.scalar.sign(dst, wt[:])
                else:
                    nc.vector.tensor_scalar(
                        dst, wt[:], 3.4e38, -1.0,
                        mybir.AluOpType.mult, mybir.AluOpType.max)
                    nc.vector.tensor_scalar(
                        dst, dst, 1.0, None, mybir.AluOpType.min)

            def transposes(mb):
                xt = xstage[mb]
                for g in range(KB // TGRP):
                    pt = psumT_pool.tile([P, TGRP, P], BF16, tag="psumT")
                    for j in range(TGRP):
                        kb = g * TGRP + j
                        nc.tensor.transpose(
                            pt[:, j, :], xt[:, kb * P:(kb + 1) * P], ident[:])
                    # Fused sign + downconvert during PSUM eviction.
                    nc.scalar.sign(xT[:, mb, g * TGRP:(g + 1) * TGRP, :], pt[:])

            def mm(po, mb, kb, nsl, start, stop):
                if USE_FP8_DR:
                    nc.tensor.matmul(
                        po[:], xT[:, mb, kb:kb + 2, :], wbin[:, kb:kb + 2, nsl],
                        start=start, stop=stop,
                        perf_mode=mybir.MatmulPerfMode.DoubleRow)
                else:
                    nc.tensor.matmul(
                        po[:], xT[:, mb, kb, :], wbin[:, kb, nsl],
                        start=start, stop=False if not stop else True)
                    nc.tensor.matmul(
                        po[:], xT[:, mb, kb + 1, :], wbin[:, kb + 1, nsl],
                        start=False, stop=stop)

            def matmuls(mb, surf=False):
                # Output staged as int16 (all values are integers <= 4096,
                # exactly representable); halves the output DMA bytes.  The
                # host widens back to f32.
                ob = obuf_pool.tile([P, N_SH], I16, tag="obuf")
                npair = KB // 2
                nsls = [slice(nb * 512, (nb + 1) * 512) for nb in range(NB)]
                pos = [psumO_pool.tile([P, 512], F32, tag="psumO", name="po")
                       for _ in range(NB)]
                if surf:
                    # Pair-major emission: each arriving w pair immediately
                    # feeds 4 matmuls (one per output chunk), so this block's
                    # matmuls overlap the w load instead of waiting for it.
                    for i in range(npair):
                        for nb in range(NB):
                            mm(pos[nb], mb, 2 * i, nsls[nb],
                               start=(i == 0), stop=(i == npair - 1))
                else:
                    for nb in range(NB):
                        for i in range(npair):
                            mm(pos[nb], mb, 2 * i, nsls[nb],
                               start=(i == 0), stop=(i == npair - 1))
                for nb in range(NB):
                    nc.vector.tensor_copy(out=ob[:, nsls[nb]], in_=pos[nb][:])
                nc.sync.dma_start(out=out[mb * P:(mb + 1) * P, :], in_=ob[:])

            # DMA issue order: a few x blocks, then all of w, then the rest
            # of x streaming behind the PE.
            for mb in range(X_HEAD):
                load_x(mb)
            for kb2 in range(KB // 2):
                load_w(kb2)
            for mb in range(X_HEAD, MB):
                load_x(mb)

            # Compute: transposes run TLOOK m-blocks ahead of matmuls.
            for mb in range(TLOOK):
                transposes(mb)
            for mb in range(MB):
                matmuls(mb, surf=(mb == 0))
                if mb + TLOOK < MB:
                    transposes(mb + TLOOK)

    nc.compile()
    return nc


def get_nc():
    global _NC_CACHE
    if _NC_CACHE is None:
        _NC_CACHE = build_nc()
    return _NC_CACHE


def kernel(x: np.ndarray, w: np.ndarray) -> np.ndarray:
    x = np.asarray(x, dtype=np.float32)
    w = np.asarray(w, dtype=np.float32)
    assert x.shape == (M_FULL, K) and w.shape == (K, N_FULL)

    nc = get_nc()
    in_maps = []
    for c in range(N_CORES):
        mi, ni = divmod(c, RN)
        in_maps.append({
            "x": np.ascontiguousarray(x[mi * M_SH:(mi + 1) * M_SH, :]),
            "w": np.ascontiguousarray(w[:, ni * N_SH:(ni + 1) * N_SH]),
        })
    res = run_bass_kernel_spmd(nc, in_maps, list(range(N_CORES)))

    out = np.empty((M_FULL, N_FULL), dtype=np.float32)
    for c in range(N_CORES):
        mi, ni = divmod(c, RN)
        out[mi * M_SH:(mi + 1) * M_SH, ni * N_SH:(ni + 1) * N_SH] = \
            res.results[c]["out"].astype(np.float32)
    return out
